# revision 4
# baseline (speedup 1.0000x reference)
"""TRN2 Bass kernel for nn_LocalAttention (B=4, T=2048, C=1024, window=16).

Sharding: 8 cores = (batch b, row-half h). Each core computes K^T/V for its
whole batch (duplicated across the 2 cores of a batch) and attention +
projections for its own 1024 rows (two 512-row chunks; h=0 gets global
chunks {0,3}, h=1 gets {1,2} for balanced structure).

All device matmuls run in fp32r (TF32-like, ~1.5e-4 rel err, 4x fp32 speed).
Orientation trick: host passes X^T and W^T so every matmul is natural:
  K^T = (Wk^T)^T @ X^T        [C, T]     (SBUF resident)
  V   = (X^T)^T @ Wv^T        [T, C]     (DRAM scratch)
  Q^T = (Wq^T)^T @ X_own^T    [C, 1024]  (DRAM scratch)
  S^T = (K^T_blk)^T @ Q^T_chunk  -> [keys, rows] so softmax-over-keys is a
        partition reduction done by a ones-vector matmul, and E^T feeds
  Y^T = V_blk^T @ E^T            [C, rows]
  Z^T = (Wo^T)^T @ Y^T           [C, rows]
Mask (j >= i - 16) applied multiplicatively post-exp, generated on-device
from per-core row/key index inputs => one uniform SPMD program.
"""
import numpy as np

import concourse.bass as bass
import concourse.mybir as mybir
import concourse.tile as tile
from concourse import bacc
from concourse import bass_utils

N_CORES = 8
B, T, C = 4, 2048, 1024
WINDOW = 16
TOWN = T // 2          # own rows per core
CHUNK = 512            # rows per processing chunk
NCHUNK = TOWN // CHUNK  # 2
CI = C // 128          # 8 contraction blocks
CO = C // 128          # 8 output blocks
KB = T // 128          # 16 key blocks
TCH = T // CHUNK       # 4 t-chunks in phase A
F32 = mybir.dt.float32
F32R = mybir.dt.float32r

_NC_CACHE = {}


def build():
    if "nc" in _NC_CACHE:
        return _NC_CACHE["nc"]
    nc = bacc.Bacc("TRN2", target_bir_lowering=False, debug=False,
                   num_devices=N_CORES)
    xt = nc.dram_tensor("xt", [C, T], F32, kind="ExternalInput").ap()
    xtq = nc.dram_tensor("xtq", [C, TOWN], F32, kind="ExternalInput").ap()
    wqt = nc.dram_tensor("wqt", [C, C], F32, kind="ExternalInput").ap()
    wkt = nc.dram_tensor("wkt", [C, C], F32, kind="ExternalInput").ap()
    wvt = nc.dram_tensor("wvt", [C, C], F32, kind="ExternalInput").ap()
    wot = nc.dram_tensor("wot", [C, C], F32, kind="ExternalInput").ap()
    keyidx16 = nc.dram_tensor("keyidx16", [128, KB], F32, kind="ExternalInput").ap()
    rowidx = nc.dram_tensor("rowidx", [1, TOWN], F32, kind="ExternalInput").ap()
    zt = nc.dram_tensor("zt", [C, TOWN], F32, kind="ExternalOutput").ap()

    xt3 = xt.rearrange("(ko ki) t -> ki ko t", ki=128)
    xtq3 = xtq.rearrange("(ko ki) t -> ki ko t", ki=128)
    w3 = {w.tensor.name: w.rearrange("(ko ki) c -> ki ko c", ki=128)
          for w in (wqt, wkt, wvt, wot)}

    inv_sqrt_c = float(1.0 / np.sqrt(C))

    with tile.TileContext(nc) as tc:
        with tc.tile_pool(name="resident", bufs=1) as res, \
             tc.tile_pool(name="dram", bufs=1, space="DRAM") as dram:
            # ---- long-lived tensors ----
            kt_sb = res.tile([128, CI, T], F32R, tag="kt")       # 64 KB/part
            v_d = dram.tile([128, KB, C], F32R)                  # V, [ki, ko, c]
            qt_d = dram.tile([128, CI, TOWN], F32R)              # Q^T
            ki16_sb = res.tile([128, KB], F32, tag="ki16")
            nc.sync.dma_start(ki16_sb[:], keyidx16[:])
            ones_row_f32 = res.tile([1, 128], F32, tag="onesrf")
            nc.vector.memset(ones_row_f32[:], 1.0)
            ones_1x128 = res.tile([1, 128], F32R, tag="o1")
            nc.vector.tensor_copy(ones_1x128[:], ones_row_f32[:])
            ones_col_f32 = res.tile([128, 1], F32, tag="onescf")
            nc.vector.memset(ones_col_f32[:], 1.0)
            ones_128x1 = res.tile([128, 1], F32R, tag="o2")
            nc.vector.tensor_copy(ones_128x1[:], ones_col_f32[:])

            # ================= Phase A1: K^T (resident) and V (DRAM) ========
            with tc.tile_pool(name="wkv", bufs=1) as wpool, \
                 tc.tile_pool(name="xa", bufs=3) as xa, \
                 tc.tile_pool(name="stage_a", bufs=3) as stage, \
                 tc.tile_pool(name="ps_a", bufs=3, space="PSUM") as ps_a, \
                 tc.tile_pool(name="ps_v", bufs=2, space="PSUM") as ps_v:
                wk_sb = wpool.tile([128, CI, C], F32R, tag="wk")
                nc.gpsimd.dma_start(wk_sb[:], w3["wkt"])
                wv_sb = wpool.tile([128, CI, C], F32R, tag="wv")
                nc.gpsimd.dma_start(wv_sb[:], w3["wvt"])
                for tch in range(TCH):
                    xt_sb = xa.tile([128, CI, CHUNK], F32R, tag="xa")
                    nc.gpsimd.dma_start(
                        xt_sb[:], xt3[:, :, tch * CHUNK:(tch + 1) * CHUNK])
                    # K^T [cout, t]: accumulate over ci
                    for co in range(CO):
                        kps = ps_a.tile([128, CHUNK], F32, tag="kps")
                        for ci in range(CI):
                            nc.tensor.matmul(
                                kps[:], wk_sb[:, ci, co * 128:(co + 1) * 128],
                                xt_sb[:, ci, :], start=(ci == 0), stop=(ci == CI - 1))
                        nc.vector.tensor_copy(
                            kt_sb[:, co, tch * CHUNK:(tch + 1) * CHUNK], kps[:])
                    # V [t, cout]: per 128-row block, accumulate over ci
                    for tb in range(CHUNK // 128):
                        vstage = stage.tile([128, C], F32R, tag="vstage")
                        for half in range(2):
                            vps = ps_v.tile([128, 512], F32, tag="vps")
                            for ci in range(CI):
                                nc.tensor.matmul(
                                    vps[:], xt_sb[:, ci, tb * 128:(tb + 1) * 128],
                                    wv_sb[:, ci, half * 512:(half + 1) * 512],
                                    start=(ci == 0), stop=(ci == CI - 1))
                            nc.vector.tensor_copy(
                                vstage[:, half * 512:(half + 1) * 512], vps[:])
                        nc.sync.dma_start(
                            v_d[:, tch * (CHUNK // 128) + tb, :], vstage[:])

            # ================= Phase A2: Q^T (DRAM) =========================
            with tc.tile_pool(name="wq", bufs=1) as wqpool, \
                 tc.tile_pool(name="xq", bufs=3) as xq, \
                 tc.tile_pool(name="stage_q", bufs=3) as stage_q, \
                 tc.tile_pool(name="ps_q", bufs=3, space="PSUM") as ps_q:
                wq_sb = wqpool.tile([128, CI, C], F32R, tag="wq")
                nc.gpsimd.dma_start(wq_sb[:], w3["wqt"])
                for tch in range(TOWN // CHUNK):
                    xq_sb = xq.tile([128, CI, CHUNK], F32R, tag="xq")
                    nc.gpsimd.dma_start(
                        xq_sb[:], xtq3[:, :, tch * CHUNK:(tch + 1) * CHUNK])
                    for co in range(CO):
                        qps = ps_q.tile([128, CHUNK], F32, tag="qps")
                        for ci in range(CI):
                            nc.tensor.matmul(
                                qps[:], wq_sb[:, ci, co * 128:(co + 1) * 128],
                                xq_sb[:, ci, :], start=(ci == 0), stop=(ci == CI - 1))
                        qstage = stage_q.tile([128, CHUNK], F32R, tag="qstage")
                        nc.vector.tensor_copy(qstage[:], qps[:])
                        nc.sync.dma_start(
                            qt_d[:, co, tch * CHUNK:(tch + 1) * CHUNK], qstage[:])

            # ================= Phase B: attention + out-proj ================
            with tc.tile_pool(name="wo", bufs=1) as wopool, \
                 tc.tile_pool(name="et", bufs=1) as etp, \
                 tc.tile_pool(name="qtc", bufs=1) as qtc_p, \
                 tc.tile_pool(name="vstream", bufs=2) as vsp, \
                 tc.tile_pool(name="ysb", bufs=1) as ysb_p, \
                 tc.tile_pool(name="workb", bufs=2) as wb, \
                 tc.tile_pool(name="zst", bufs=2) as zstp, \
                 tc.tile_pool(name="ps_s", bufs=2, space="PSUM") as ps_s, \
                 tc.tile_pool(name="ps_acc", bufs=1, space="PSUM") as ps_acc, \
                 tc.tile_pool(name="ps_bc", bufs=1, space="PSUM") as ps_bc, \
                 tc.tile_pool(name="ps_y", bufs=2, space="PSUM") as ps_y, \
                 tc.tile_pool(name="ps_z", bufs=2, space="PSUM") as ps_z:
                wo_sb = wopool.tile([128, CI, C], F32R, tag="wo")
                nc.gpsimd.dma_start(wo_sb[:], w3["wot"])
                for ch in range(NCHUNK):
                    rsl = slice(ch * CHUNK, (ch + 1) * CHUNK)
                    qt_c = qtc_p.tile([128, CI, CHUNK], F32R, tag="qtc")
                    nc.sync.dma_start(qt_c[:], qt_d[:, :, rsl])
                    # rowidx broadcast to [128, CHUNK] via K=1 matmul
                    ri_sb = wb.tile([1, CHUNK], F32R, tag="ri")
                    nc.gpsimd.dma_start(ri_sb[:], rowidx[:, rsl])
                    ri_ps = ps_bc.tile([128, CHUNK], F32, tag="bcast")
                    nc.tensor.matmul(ri_ps[:], ones_1x128[:], ri_sb[:],
                                     start=True, stop=True)
                    ri_b = wb.tile([128, CHUNK], F32, tag="rib")
                    nc.vector.tensor_copy(ri_b[:], ri_ps[:])

                    et = etp.tile([128, KB, CHUNK], F32R, tag="et")
                    # --- sweep 1a: scores + exp + mask ---
                    for kb in range(KB):
                        sps = ps_s.tile([128, CHUNK], F32, tag="sps")
                        for ci in range(CI):
                            nc.tensor.matmul(
                                sps[:], kt_sb[:, ci, kb * 128:(kb + 1) * 128],
                                qt_c[:, ci, :], start=(ci == 0), stop=(ci == CI - 1))
                        nc.scalar.activation(et[:, kb, :], sps[:],
                                             mybir.ActivationFunctionType.Exp,
                                             scale=inv_sqrt_c)
                        mask = wb.tile([128, CHUNK], F32, tag="mask")
                        nc.vector.tensor_tensor(
                            mask[:], ki16_sb[:, kb:kb + 1].to_broadcast((128, CHUNK)),
                            ri_b[:], mybir.AluOpType.is_ge)
                        nc.vector.tensor_tensor(et[:, kb, :], et[:, kb, :], mask[:],
                                                mybir.AluOpType.mult)
                    # --- sweep 1b: key-sums via ones matmul ---
                    sums_ps = ps_acc.tile([1, CHUNK], F32, tag="sums")
                    for kb in range(KB):
                        nc.tensor.matmul(sums_ps[:], ones_128x1[:], et[:, kb, :],
                                         start=(kb == 0), stop=(kb == KB - 1))
                    recip = wb.tile([1, CHUNK], F32R, tag="recip")
                    with nc.allow_low_precision(reason="fp32r normalizer broadcast"):
                        nc.vector.reciprocal(recip[:], sums_ps[:])
                    rb_ps = ps_bc.tile([128, CHUNK], F32, tag="bcast")
                    nc.tensor.matmul(rb_ps[:], ones_1x128[:], recip[:],
                                     start=True, stop=True)
                    rb_sb = wb.tile([128, CHUNK], F32, tag="rbsb")
                    nc.vector.tensor_copy(rb_sb[:], rb_ps[:])

                    # --- sweep 2: Y^T = V^T @ E^T per cout block ---
                    y_sb = ysb_p.tile([128, CO, CHUNK], F32R, tag="ysb")
                    for co in range(CO):
                        v_co = vsp.tile([128, KB, 128], F32R, tag="vco")
                        nc.sync.dma_start(v_co[:], v_d[:, :, co * 128:(co + 1) * 128])
                        yps = ps_y.tile([128, CHUNK], F32, tag="yps")
                        for kb in range(KB):
                            nc.tensor.matmul(yps[:], v_co[:, kb, :], et[:, kb, :],
                                             start=(kb == 0), stop=(kb == KB - 1))
                        nc.vector.tensor_copy(y_sb[:, co, :], yps[:])

                    # --- out-proj + normalize ---
                    for co in range(CO):
                        zps = ps_z.tile([128, CHUNK], F32, tag="zps")
                        for ci in range(CI):
                            nc.tensor.matmul(
                                zps[:], wo_sb[:, ci, co * 128:(co + 1) * 128],
                                y_sb[:, ci, :], start=(ci == 0), stop=(ci == CI - 1))
                        zst = zstp.tile([128, CHUNK], F32, tag="zst")
                        nc.vector.tensor_tensor(zst[:], zps[:], rb_sb[:],
                                                mybir.AluOpType.mult)
                        nc.sync.dma_start(zt[co * 128:(co + 1) * 128, rsl], zst[:])
    nc.compile()
    _NC_CACHE["nc"] = nc
    return nc


def make_in_maps(inputs):
    x = np.asarray(inputs["x"], dtype=np.float32)
    for bname in ("bq", "bk", "bv", "bo"):
        bval = np.asarray(inputs[bname])
        assert np.all(bval == 0.0), f"{bname} nonzero: unsupported fast path"
    wqt = np.ascontiguousarray(np.asarray(inputs["Wq"], np.float32).T)
    wkt = np.ascontiguousarray(np.asarray(inputs["Wk"], np.float32).T)
    wvt = np.ascontiguousarray(np.asarray(inputs["Wv"], np.float32).T)
    wot = np.ascontiguousarray(np.asarray(inputs["Wo"], np.float32).T)
    keyidx16 = (np.arange(T, dtype=np.float32).reshape(KB, 128).T + WINDOW
                ).copy()  # [128, KB]
    chunk_map = {0: (0, 3), 1: (1, 2)}
    in_maps = []
    for core in range(N_CORES):
        b, h = divmod(core, 2)
        xt_b = np.ascontiguousarray(x[b].T)  # [C, T]
        ch0, ch1 = chunk_map[h]
        xtq = np.concatenate(
            [xt_b[:, ch0 * CHUNK:(ch0 + 1) * CHUNK],
             xt_b[:, ch1 * CHUNK:(ch1 + 1) * CHUNK]], axis=1)
        rowidx = np.concatenate(
            [np.arange(ch0 * CHUNK, (ch0 + 1) * CHUNK, dtype=np.float32),
             np.arange(ch1 * CHUNK, (ch1 + 1) * CHUNK, dtype=np.float32)]
        ).reshape(1, TOWN)
        in_maps.append({
            "xt": xt_b, "xtq": np.ascontiguousarray(xtq),
            "wqt": wqt, "wkt": wkt, "wvt": wvt, "wot": wot,
            "keyidx16": keyidx16, "rowidx": rowidx,
        })
    return in_maps


def gather_output(results, dtype):
    out = np.empty((B, T, C), dtype=dtype)
    chunk_map = {0: (0, 3), 1: (1, 2)}
    for core in range(N_CORES):
        b, h = divmod(core, 2)
        y = results[core]["zt"].T  # [TOWN rows, C]
        ch0, ch1 = chunk_map[h]
        out[b, ch0 * CHUNK:(ch0 + 1) * CHUNK] = y[:CHUNK]
        out[b, ch1 * CHUNK:(ch1 + 1) * CHUNK] = y[CHUNK:]
    return out


def kernel(**inputs):
    nc = build()
    in_maps = make_in_maps(inputs)
    res = bass_utils.run_bass_kernel_spmd(nc, in_maps,
                                          core_ids=list(range(N_CORES)))
    return gather_output(res.results, np.asarray(inputs["x"]).dtype)


# revision 5
# speedup vs baseline: 1.0038x; 1.0038x over previous
"""TRN2 Bass kernel for nn_LocalAttention (B=4, T=2048, C=1024, window=16).

Sharding: 8 cores = (batch b, row-half h). Each core computes K^T/V for its
whole batch (duplicated across the 2 cores of a batch) and attention +
projections for its own 1024 rows (two 512-row chunks; h=0 gets global
chunks {0,3}, h=1 gets {1,2}).

All matmuls run in fp32r (TF32-like, ~1.5e-4 rel err, 4x fp32 speed). Raw
fp32 bytes are declared as fp32r at the DRAM boundary - the PE rounds
internally (validated: identical error to explicit cast-DMA).

Orientation trick: host passes X^T and W^T so every matmul is natural:
  K^T = (Wk^T)^T @ X^T        [C, T]     (DRAM scratch)
  V   = (X^T)^T @ Wv^T        [T, C]     (DRAM scratch)
  Q^T = (Wq^T)^T @ X_own^T    [C, 1024]  (DRAM scratch)
  S^T = (K^T_blk)^T @ Q^T_chunk  -> [keys, rows]; softmax-over-keys is a
        partition reduction done by a ones-vector matmul, and E^T feeds
  Y^T = V_blk^T @ E^T            [C, rows]
  Z^T = (Wo^T)^T @ Y^T           [C, rows]
Mask (j >= i - 16) applied multiplicatively post-exp, generated on-device
from per-core row/key index inputs (host-broadcast rowidx, exact fp32)
=> one uniform SPMD program across all 8 cores.
"""
import numpy as np

import concourse.bass as bass
import concourse.mybir as mybir
import concourse.tile as tile
from concourse import bacc
from concourse import bass_utils

N_CORES = 8
B, T, C = 4, 2048, 1024
WINDOW = 16
TOWN = T // 2          # own rows per core
CHUNK = 512            # rows per processing chunk
NCHUNK = TOWN // CHUNK  # 2
CI = C // 128          # 8 contraction blocks
CO = C // 128          # 8 output blocks
KB = T // 128          # 16 key blocks
TCH = T // CHUNK       # 4 t-chunks in phase A
F32 = mybir.dt.float32
F32R = mybir.dt.float32r

_NC_CACHE = {}


def build():
    if "nc" in _NC_CACHE:
        return _NC_CACHE["nc"]
    nc = bacc.Bacc("TRN2", target_bir_lowering=False, debug=False,
                   num_devices=N_CORES)
    xt = nc.dram_tensor("xt", [C, T], F32R, kind="ExternalInput").ap()
    xtq = nc.dram_tensor("xtq", [C, TOWN], F32R, kind="ExternalInput").ap()
    wqt = nc.dram_tensor("wqt", [C, C], F32R, kind="ExternalInput").ap()
    wkt = nc.dram_tensor("wkt", [C, C], F32R, kind="ExternalInput").ap()
    wvt = nc.dram_tensor("wvt", [C, C], F32R, kind="ExternalInput").ap()
    wot = nc.dram_tensor("wot", [C, C], F32R, kind="ExternalInput").ap()
    keyidx16 = nc.dram_tensor("keyidx16", [128, KB], F32, kind="ExternalInput").ap()
    rowidxb = nc.dram_tensor("rowidxb", [128, TOWN], F32, kind="ExternalInput").ap()
    zt = nc.dram_tensor("zt", [C, TOWN], F32, kind="ExternalOutput").ap()

    xt3 = xt.rearrange("(ko ki) t -> ki ko t", ki=128)
    xtq3 = xtq.rearrange("(ko ki) t -> ki ko t", ki=128)
    w3 = {w.tensor.name: w.rearrange("(ko ki) c -> ki ko c", ki=128)
          for w in (wqt, wkt, wvt, wot)}

    inv_sqrt_c = float(1.0 / np.sqrt(C))

    with tile.TileContext(nc) as tc:
        with tc.tile_pool(name="res", bufs=1) as res, \
             tc.tile_pool(name="dram", bufs=1, space="DRAM") as dram:
            kt_d = dram.tile([128, CI, T], F32R)      # K^T  [ki, ko, t]
            v_d = dram.tile([128, KB, C], F32R)       # V    [ki, ko, c]
            qt_d = dram.tile([128, CI, TOWN], F32R)   # Q^T  [ki, ko, t]
            # long-lived small tensors + wo (loaded early, used late)
            wo_sb = res.tile([128, CI, C], F32R, tag="wo")
            for ci in range(CI):
                nc.scalar.dma_start(wo_sb[:, ci, :], w3["wot"][:, ci, :])
            ki16_sb = res.tile([128, KB], F32, tag="ki16")
            nc.sync.dma_start(ki16_sb[:], keyidx16[:])
            ones_row_f32 = res.tile([1, 128], F32, tag="onesrf")
            nc.vector.memset(ones_row_f32[:], 1.0)
            ones_1x128 = res.tile([1, 128], F32R, tag="o1")
            nc.vector.tensor_copy(ones_1x128[:], ones_row_f32[:])
            ones_col_f32 = res.tile([128, 1], F32, tag="onescf")
            nc.vector.memset(ones_col_f32[:], 1.0)
            ones_128x1 = res.tile([128, 1], F32R, tag="o2")
            nc.vector.tensor_copy(ones_128x1[:], ones_col_f32[:])

            # ================= Phase A: projections =========================
            with tc.tile_pool(name="wts", bufs=1) as wts, \
                 tc.tile_pool(name="xa", bufs=3) as xa, \
                 tc.tile_pool(name="stg", bufs=4) as stg, \
                 tc.tile_pool(name="ps_k", bufs=3, space="PSUM") as ps_k, \
                 tc.tile_pool(name="ps_v", bufs=2, space="PSUM") as ps_v, \
                 tc.tile_pool(name="ps_q", bufs=2, space="PSUM") as ps_q:
                wk_sb = wts.tile([128, CI, C], F32R, tag="wk")
                wv_sb = wts.tile([128, CI, C], F32R, tag="wv")
                wq_sb = wts.tile([128, CI, C], F32R, tag="wq")
                for ci in range(CI):  # split loads so compute starts early
                    nc.sync.dma_start(wk_sb[:, ci, :], w3["wkt"][:, ci, :])
                for ci in range(CI):
                    nc.sync.dma_start(wv_sb[:, ci, :], w3["wvt"][:, ci, :])
                for ci in range(CI):
                    nc.scalar.dma_start(wq_sb[:, ci, :], w3["wqt"][:, ci, :])

                for tch in range(TCH):
                    xt_sb = xa.tile([128, CI, CHUNK], F32R, tag="xa")
                    nc.sync.dma_start(
                        xt_sb[:], xt3[:, :, tch * CHUNK:(tch + 1) * CHUNK])
                    # K^T [cout, t]
                    for co in range(CO):
                        kps = ps_k.tile([128, CHUNK], F32, tag="kps")
                        for ci in range(CI):
                            nc.tensor.matmul(
                                kps[:], wk_sb[:, ci, co * 128:(co + 1) * 128],
                                xt_sb[:, ci, :], start=(ci == 0), stop=(ci == CI - 1))
                        kstage = stg.tile([128, CHUNK], F32R, tag="kstage")
                        nc.vector.tensor_copy(kstage[:], kps[:])
                        nc.sync.dma_start(
                            kt_d[:, co, tch * CHUNK:(tch + 1) * CHUNK], kstage[:])
                    # V [t, cout]
                    for tb in range(CHUNK // 128):
                        for half in range(2):
                            vps = ps_v.tile([128, 512], F32, tag="vps")
                            for ci in range(CI):
                                nc.tensor.matmul(
                                    vps[:], xt_sb[:, ci, tb * 128:(tb + 1) * 128],
                                    wv_sb[:, ci, half * 512:(half + 1) * 512],
                                    start=(ci == 0), stop=(ci == CI - 1))
                            vstage = stg.tile([128, 512], F32R, tag="vstage")
                            nc.vector.tensor_copy(vstage[:], vps[:])
                            nc.scalar.dma_start(
                                v_d[:, tch * (CHUNK // 128) + tb,
                                    half * 512:(half + 1) * 512], vstage[:])

                for qch in range(TOWN // CHUNK):
                    xq_sb = xa.tile([128, CI, CHUNK], F32R, tag="xa")
                    nc.sync.dma_start(
                        xq_sb[:], xtq3[:, :, qch * CHUNK:(qch + 1) * CHUNK])
                    for co in range(CO):
                        qps = ps_q.tile([128, CHUNK], F32, tag="qps")
                        for ci in range(CI):
                            nc.tensor.matmul(
                                qps[:], wq_sb[:, ci, co * 128:(co + 1) * 128],
                                xq_sb[:, ci, :], start=(ci == 0), stop=(ci == CI - 1))
                        qstage = stg.tile([128, CHUNK], F32R, tag="qstage")
                        nc.vector.tensor_copy(qstage[:], qps[:])
                        nc.sync.dma_start(
                            qt_d[:, co, qch * CHUNK:(qch + 1) * CHUNK], qstage[:])

            # ================= Phase B: attention + out-proj ================
            with tc.tile_pool(name="et", bufs=1) as etp, \
                 tc.tile_pool(name="qtc", bufs=2) as qtc_p, \
                 tc.tile_pool(name="ktb", bufs=4) as ktb_p, \
                 tc.tile_pool(name="vco", bufs=3) as vsp, \
                 tc.tile_pool(name="ysb", bufs=2) as ysb_p, \
                 tc.tile_pool(name="wb", bufs=2) as wb, \
                 tc.tile_pool(name="zst", bufs=3) as zstp, \
                 tc.tile_pool(name="ps_s", bufs=2, space="PSUM") as ps_s, \
                 tc.tile_pool(name="ps_acc", bufs=1, space="PSUM") as ps_acc, \
                 tc.tile_pool(name="ps_bc", bufs=1, space="PSUM") as ps_bc, \
                 tc.tile_pool(name="ps_y", bufs=2, space="PSUM") as ps_y, \
                 tc.tile_pool(name="ps_z", bufs=2, space="PSUM") as ps_z:
                for ch in range(NCHUNK):
                    rsl = slice(ch * CHUNK, (ch + 1) * CHUNK)
                    qt_c = qtc_p.tile([128, CI, CHUNK], F32R, tag="qtc")
                    nc.sync.dma_start(qt_c[:], qt_d[:, :, rsl])
                    ri_b = wb.tile([128, CHUNK], F32, tag="rib")
                    nc.sync.dma_start(ri_b[:], rowidxb[:, rsl])

                    et = etp.tile([128, KB, CHUNK], F32R, tag="et")
                    # --- sweep 1a: scores + exp + mask ---
                    for kb in range(KB):
                        kt_b = ktb_p.tile([128, CI, 128], F32R, tag="ktb")
                        nc.scalar.dma_start(
                            kt_b[:], kt_d[:, :, kb * 128:(kb + 1) * 128])
                        sps = ps_s.tile([128, CHUNK], F32, tag="sps")
                        for ci in range(CI):
                            nc.tensor.matmul(
                                sps[:], kt_b[:, ci, :], qt_c[:, ci, :],
                                start=(ci == 0), stop=(ci == CI - 1))
                        nc.scalar.activation(et[:, kb, :], sps[:],
                                             mybir.ActivationFunctionType.Exp,
                                             scale=inv_sqrt_c)
                        mask = wb.tile([128, CHUNK], F32, tag="mask")
                        nc.vector.tensor_tensor(
                            mask[:], ki16_sb[:, kb:kb + 1].to_broadcast((128, CHUNK)),
                            ri_b[:], mybir.AluOpType.is_ge)
                        nc.vector.tensor_tensor(et[:, kb, :], et[:, kb, :], mask[:],
                                                mybir.AluOpType.mult)
                    # --- sweep 1b: key-sums via ones matmul ---
                    sums_ps = ps_acc.tile([1, CHUNK], F32, tag="sums")
                    for kb in range(KB):
                        nc.tensor.matmul(sums_ps[:], ones_128x1[:], et[:, kb, :],
                                         start=(kb == 0), stop=(kb == KB - 1))
                    recip = wb.tile([1, CHUNK], F32R, tag="recip")
                    with nc.allow_low_precision(reason="fp32r normalizer broadcast"):
                        nc.vector.reciprocal(recip[:], sums_ps[:])
                    rb_ps = ps_bc.tile([128, CHUNK], F32, tag="bcast")
                    nc.tensor.matmul(rb_ps[:], ones_1x128[:], recip[:],
                                     start=True, stop=True)
                    rb_sb = wb.tile([128, CHUNK], F32, tag="rbsb")
                    nc.vector.tensor_copy(rb_sb[:], rb_ps[:])

                    # --- sweep 2: Y^T = V^T @ E^T per cout block ---
                    y_sb = ysb_p.tile([128, CO, CHUNK], F32R, tag="ysb")
                    for co in range(CO):
                        v_co = vsp.tile([128, KB, 128], F32R, tag="vco")
                        nc.sync.dma_start(v_co[:], v_d[:, :, co * 128:(co + 1) * 128])
                        yps = ps_y.tile([128, CHUNK], F32, tag="yps")
                        for kb in range(KB):
                            nc.tensor.matmul(yps[:], v_co[:, kb, :], et[:, kb, :],
                                             start=(kb == 0), stop=(kb == KB - 1))
                        nc.vector.tensor_copy(y_sb[:, co, :], yps[:])

                    # --- out-proj + normalize ---
                    for co in range(CO):
                        zps = ps_z.tile([128, CHUNK], F32, tag="zps")
                        for ci in range(CI):
                            nc.tensor.matmul(
                                zps[:], wo_sb[:, ci, co * 128:(co + 1) * 128],
                                y_sb[:, ci, :], start=(ci == 0), stop=(ci == CI - 1))
                        zst = zstp.tile([128, CHUNK], F32, tag="zst")
                        nc.vector.tensor_tensor(zst[:], zps[:], rb_sb[:],
                                                mybir.AluOpType.mult)
                        nc.sync.dma_start(zt[co * 128:(co + 1) * 128, rsl], zst[:])
    nc.compile()
    _NC_CACHE["nc"] = nc
    return nc


def make_in_maps(inputs):
    x = np.asarray(inputs["x"], dtype=np.float32)
    for bname in ("bq", "bk", "bv", "bo"):
        bval = np.asarray(inputs[bname])
        assert np.all(bval == 0.0), f"{bname} nonzero: unsupported fast path"
    wqt = np.ascontiguousarray(np.asarray(inputs["Wq"], np.float32).T)
    wkt = np.ascontiguousarray(np.asarray(inputs["Wk"], np.float32).T)
    wvt = np.ascontiguousarray(np.asarray(inputs["Wv"], np.float32).T)
    wot = np.ascontiguousarray(np.asarray(inputs["Wo"], np.float32).T)
    keyidx16 = (np.arange(T, dtype=np.float32).reshape(KB, 128).T + WINDOW
                ).copy()  # [128, KB]
    chunk_map = {0: (0, 3), 1: (1, 2)}
    in_maps = []
    for core in range(N_CORES):
        b, h = divmod(core, 2)
        xt_b = np.ascontiguousarray(x[b].T)  # [C, T]
        ch0, ch1 = chunk_map[h]
        xtq = np.concatenate(
            [xt_b[:, ch0 * CHUNK:(ch0 + 1) * CHUNK],
             xt_b[:, ch1 * CHUNK:(ch1 + 1) * CHUNK]], axis=1)
        rowidx = np.concatenate(
            [np.arange(ch0 * CHUNK, (ch0 + 1) * CHUNK, dtype=np.float32),
             np.arange(ch1 * CHUNK, (ch1 + 1) * CHUNK, dtype=np.float32)])
        rowidxb = np.ascontiguousarray(
            np.broadcast_to(rowidx[None, :], (128, TOWN)))
        in_maps.append({
            "xt": xt_b, "xtq": np.ascontiguousarray(xtq),
            "wqt": wqt, "wkt": wkt, "wvt": wvt, "wot": wot,
            "keyidx16": keyidx16, "rowidxb": rowidxb,
        })
    return in_maps


def gather_output(results, dtype):
    out = np.empty((B, T, C), dtype=dtype)
    chunk_map = {0: (0, 3), 1: (1, 2)}
    for core in range(N_CORES):
        b, h = divmod(core, 2)
        y = results[core]["zt"].T  # [TOWN rows, C]
        ch0, ch1 = chunk_map[h]
        out[b, ch0 * CHUNK:(ch0 + 1) * CHUNK] = y[:CHUNK]
        out[b, ch1 * CHUNK:(ch1 + 1) * CHUNK] = y[CHUNK:]
    return out


def kernel(**inputs):
    nc = build()
    in_maps = make_in_maps(inputs)
    res = bass_utils.run_bass_kernel_spmd(nc, in_maps,
                                          core_ids=list(range(N_CORES)))
    return gather_output(res.results, np.asarray(inputs["x"]).dtype)


# revision 7
# speedup vs baseline: 1.1970x; 1.1925x over previous
"""TRN2 Bass kernel for nn_LocalAttention (B=4, T=2048, C=1024, window=16).

Sharding: 8 cores = (batch b, row-half h). Each core computes K^T/V for its
whole batch (duplicated across the 2 cores of a batch) and attention +
projections for its own 1024 rows (two 512-row chunks; h=0 gets global
chunks {0,3}, h=1 gets {1,2}; slot 0 = denser chunk).

All matmuls run in fp32r (TF32-like, ~1.5e-4 rel err, 4x fp32 speed). Raw
fp32 bytes are declared as fp32r at the DRAM boundary - the PE rounds
internally (validated: identical error to explicit cast-DMA).

Orientation trick: host passes X^T and W^T so every matmul is natural:
  K^T = (Wk^T)^T @ X^T        [C, T]     (DRAM scratch)
  V   = (X^T)^T @ Wv^T        [T, C]     (DRAM scratch)
  Q^T = (Wq^T)^T @ X_own^T    [C, 1024]  (SBUF resident)
  S^T = (K^T_blk)^T @ Q^T_chunk  -> [keys, rows]; softmax-over-keys is a
        partition reduction done by a ones-vector matmul, and E^T feeds
  Y^T = V_blk^T @ E^T            [C, rows]
  Z^T = (Wo^T)^T @ Y^T           [C, rows]

Sparsity: mask keeps j >= i - 16 (reverse-causal), so each 512-row chunk's
kept key-block set is a SUFFIX {b..15}; processing key blocks in descending
order (position p -> block 15-p) makes every kept set a static PREFIX.
Chunk slot 0 runs 16 positions, slot 1 runs 9 - uniform across cores, the
data-driven is_ge mask zeroes over-included blocks. Mask applied
multiplicatively post-exp (scores are O(6), no overflow without max-sub).
"""
import numpy as np

import concourse.bass as bass
import concourse.mybir as mybir
import concourse.tile as tile
from concourse import bacc
from concourse import bass_utils

N_CORES = 8
B, T, C = 4, 2048, 1024
WINDOW = 16
TOWN = T // 2          # own rows per core
CHUNK = 512            # rows per processing chunk
NCHUNK = TOWN // CHUNK  # 2
CI = C // 128          # 8 contraction blocks
CO = C // 128          # 8 output blocks
KB = T // 128          # 16 key blocks
TCH = T // CHUNK       # 4 t-chunks in phase A
SLOT_KBS = (16, 9)     # key-block positions per chunk slot (descending order)
F32 = mybir.dt.float32
F32R = mybir.dt.float32r

_NC_CACHE = {}


def build():
    if "nc" in _NC_CACHE:
        return _NC_CACHE["nc"]
    nc = bacc.Bacc("TRN2", target_bir_lowering=False, debug=False,
                   num_devices=N_CORES)
    xt = nc.dram_tensor("xt", [C, T], F32R, kind="ExternalInput").ap()
    xtq = nc.dram_tensor("xtq", [C, TOWN], F32R, kind="ExternalInput").ap()
    wqt = nc.dram_tensor("wqt", [C, C], F32R, kind="ExternalInput").ap()
    wkt = nc.dram_tensor("wkt", [C, C], F32R, kind="ExternalInput").ap()
    wvt = nc.dram_tensor("wvt", [C, C], F32R, kind="ExternalInput").ap()
    wot = nc.dram_tensor("wot", [C, C], F32R, kind="ExternalInput").ap()
    keyidx16 = nc.dram_tensor("keyidx16", [128, KB], F32, kind="ExternalInput").ap()
    rowidxb = nc.dram_tensor("rowidxb", [128, TOWN], F32, kind="ExternalInput").ap()
    zt = nc.dram_tensor("zt", [C, TOWN], F32, kind="ExternalOutput").ap()

    xt3 = xt.rearrange("(ko ki) t -> ki ko t", ki=128)
    xtq3 = xtq.rearrange("(ko ki) t -> ki ko t", ki=128)
    w3 = {w.tensor.name: w.rearrange("(ko ki) c -> ki ko c", ki=128)
          for w in (wqt, wkt, wvt, wot)}

    inv_sqrt_c = float(1.0 / np.sqrt(C))

    with tile.TileContext(nc) as tc:
        with tc.tile_pool(name="res", bufs=1) as res, \
             tc.tile_pool(name="dram", bufs=1, space="DRAM") as dram:
            kt_d = dram.tile([128, CI, T], F32R)      # K^T  [ki, ko, t]
            v_d = dram.tile([128, KB, C], F32R)       # V    [ki, ko, c]
            qt_sb = res.tile([128, CI, TOWN], F32R, tag="qt")  # Q^T resident
            wo_sb = res.tile([128, CI, C], F32R, tag="wo")
            ki16_sb = res.tile([128, KB], F32, tag="ki16")
            nc.gpsimd.dma_start(ki16_sb[:], keyidx16[:])
            ones_row_f32 = res.tile([1, 128], F32, tag="onesrf")
            nc.vector.memset(ones_row_f32[:], 1.0)
            ones_1x128 = res.tile([1, 128], F32R, tag="o1")
            nc.vector.tensor_copy(ones_1x128[:], ones_row_f32[:])
            ones_col_f32 = res.tile([128, 1], F32, tag="onescf")
            nc.vector.memset(ones_col_f32[:], 1.0)
            ones_128x1 = res.tile([128, 1], F32R, tag="o2")
            nc.vector.tensor_copy(ones_128x1[:], ones_col_f32[:])

            # ================= Phase A: projections =========================
            with tc.tile_pool(name="wts", bufs=1) as wts, \
                 tc.tile_pool(name="xa", bufs=2) as xa, \
                 tc.tile_pool(name="stg", bufs=3) as stg, \
                 tc.tile_pool(name="ps_k", bufs=3, space="PSUM") as ps_k, \
                 tc.tile_pool(name="ps_v", bufs=2, space="PSUM") as ps_v, \
                 tc.tile_pool(name="ps_q", bufs=2, space="PSUM") as ps_q:
                wk_sb = wts.tile([128, CI, C], F32R, tag="wk")
                wv_sb = wts.tile([128, CI, C], F32R, tag="wv")
                wq_sb = wts.tile([128, CI, C], F32R, tag="wq")
                # first xt chunk before anything else on the sync queue
                xt_sbs = []
                xt_sb0 = xa.tile([128, CI, CHUNK], F32R, tag="xa")
                nc.sync.dma_start(xt_sb0[:], xt3[:, :, 0:CHUNK])
                for ci in range(CI):
                    nc.sync.dma_start(wk_sb[:, ci, :], w3["wkt"][:, ci, :])
                for ci in range(CI):
                    nc.scalar.dma_start(wv_sb[:, ci, :], w3["wvt"][:, ci, :])
                for ci in range(CI):
                    nc.scalar.dma_start(wq_sb[:, ci, :], w3["wqt"][:, ci, :])

                for tch in range(TCH):
                    if tch == 0:
                        xt_sb = xt_sb0
                    else:
                        xt_sb = xa.tile([128, CI, CHUNK], F32R, tag="xa")
                        nc.sync.dma_start(
                            xt_sb[:], xt3[:, :, tch * CHUNK:(tch + 1) * CHUNK])
                    # K^T [cout, t]
                    for co in range(CO):
                        kps = ps_k.tile([128, CHUNK], F32, tag="kps")
                        for ci in range(CI):
                            nc.tensor.matmul(
                                kps[:], wk_sb[:, ci, co * 128:(co + 1) * 128],
                                xt_sb[:, ci, :], start=(ci == 0), stop=(ci == CI - 1))
                        kstage = stg.tile([128, CHUNK], F32R, tag="kstage")
                        nc.vector.tensor_copy(kstage[:], kps[:])
                        nc.sync.dma_start(
                            kt_d[:, co, tch * CHUNK:(tch + 1) * CHUNK], kstage[:])
                    # V [t, cout]
                    for tb in range(CHUNK // 128):
                        for half in range(2):
                            vps = ps_v.tile([128, 512], F32, tag="vps")
                            for ci in range(CI):
                                nc.tensor.matmul(
                                    vps[:], xt_sb[:, ci, tb * 128:(tb + 1) * 128],
                                    wv_sb[:, ci, half * 512:(half + 1) * 512],
                                    start=(ci == 0), stop=(ci == CI - 1))
                            vstage = stg.tile([128, 512], F32R, tag="vstage")
                            nc.vector.tensor_copy(vstage[:], vps[:])
                            nc.scalar.dma_start(
                                v_d[:, tch * (CHUNK // 128) + tb,
                                    half * 512:(half + 1) * 512], vstage[:])

                for qch in range(TOWN // CHUNK):
                    xq_sb = xa.tile([128, CI, CHUNK], F32R, tag="xa")
                    nc.sync.dma_start(
                        xq_sb[:], xtq3[:, :, qch * CHUNK:(qch + 1) * CHUNK])
                    for co in range(CO):
                        qps = ps_q.tile([128, CHUNK], F32, tag="qps")
                        for ci in range(CI):
                            nc.tensor.matmul(
                                qps[:], wq_sb[:, ci, co * 128:(co + 1) * 128],
                                xq_sb[:, ci, :], start=(ci == 0), stop=(ci == CI - 1))
                        nc.vector.tensor_copy(
                            qt_sb[:, co, qch * CHUNK:(qch + 1) * CHUNK], qps[:])

            # wo load late on the scalar queue (used only at Z, end of B)
            for ci in range(CI):
                nc.scalar.dma_start(wo_sb[:, ci, :], w3["wot"][:, ci, :])

            # ================= Phase B: attention + out-proj ================
            with tc.tile_pool(name="et", bufs=1) as etp, \
                 tc.tile_pool(name="ktb", bufs=4) as ktb_p, \
                 tc.tile_pool(name="vco", bufs=3) as vsp, \
                 tc.tile_pool(name="ysb", bufs=2) as ysb_p, \
                 tc.tile_pool(name="wb", bufs=2) as wb, \
                 tc.tile_pool(name="zst", bufs=3) as zstp, \
                 tc.tile_pool(name="ps_s", bufs=3, space="PSUM") as ps_s, \
                 tc.tile_pool(name="ps_sh", bufs=1, space="PSUM") as ps_sh, \
                 tc.tile_pool(name="ps_y", bufs=2, space="PSUM") as ps_y, \
                 tc.tile_pool(name="ps_z", bufs=2, space="PSUM") as ps_z:
                for ch in range(NCHUNK):
                    nkb = SLOT_KBS[ch]
                    rsl = slice(ch * CHUNK, (ch + 1) * CHUNK)
                    ri_b = wb.tile([128, CHUNK], F32, tag="rib")
                    nc.sync.dma_start(ri_b[:], rowidxb[:, rsl])

                    et = etp.tile([128, KB, CHUNK], F32R, tag="et")
                    # --- sweep 1a: scores + exp + mask (descending kb) ---
                    for p in range(nkb):
                        kb = KB - 1 - p
                        kt_b = ktb_p.tile([128, CI, 128], F32R, tag="ktb")
                        nc.scalar.dma_start(
                            kt_b[:], kt_d[:, :, kb * 128:(kb + 1) * 128])
                        sps = ps_s.tile([128, CHUNK], F32, tag="sps")
                        for ci in range(CI):
                            nc.tensor.matmul(
                                sps[:], kt_b[:, ci, :], qt_sb[:, ci, rsl],
                                start=(ci == 0), stop=(ci == CI - 1))
                        nc.scalar.activation(et[:, p, :], sps[:],
                                             mybir.ActivationFunctionType.Exp,
                                             scale=inv_sqrt_c)
                        mask = wb.tile([128, CHUNK], F32, tag="mask")
                        nc.vector.tensor_tensor(
                            mask[:], ki16_sb[:, kb:kb + 1].to_broadcast((128, CHUNK)),
                            ri_b[:], mybir.AluOpType.is_ge)
                        nc.vector.tensor_tensor(et[:, p, :], et[:, p, :], mask[:],
                                                mybir.AluOpType.mult)
                    # --- sweep 1b: key-sums via ones matmul ---
                    sums_ps = ps_sh.tile([1, CHUNK], F32, tag="shared")
                    for p in range(nkb):
                        nc.tensor.matmul(sums_ps[:], ones_128x1[:], et[:, p, :],
                                         start=(p == 0), stop=(p == nkb - 1))
                    recip = wb.tile([1, CHUNK], F32R, tag="recip")
                    with nc.allow_low_precision(reason="fp32r normalizer broadcast"):
                        nc.vector.reciprocal(recip[:], sums_ps[:])
                    rb_ps = ps_sh.tile([128, CHUNK], F32, tag="shared")
                    nc.tensor.matmul(rb_ps[:], ones_1x128[:], recip[:],
                                     start=True, stop=True)
                    rb_sb = wb.tile([128, CHUNK], F32, tag="rbsb")
                    nc.vector.tensor_copy(rb_sb[:], rb_ps[:])

                    # --- sweep 2: Y^T = V^T @ E^T per cout block ---
                    y_sb = ysb_p.tile([128, CO, CHUNK], F32R, tag="ysb")
                    for co in range(CO):
                        v_co = vsp.tile([128, KB, 128], F32R, tag="vco")
                        nc.sync.dma_start(
                            v_co[:, :nkb, :],
                            v_d[:, KB - nkb:, co * 128:(co + 1) * 128])
                        yps = ps_y.tile([128, CHUNK], F32, tag="yps")
                        for p in range(nkb):
                            nc.tensor.matmul(yps[:], v_co[:, nkb - 1 - p, :],
                                             et[:, p, :],
                                             start=(p == 0), stop=(p == nkb - 1))
                        nc.vector.tensor_copy(y_sb[:, co, :], yps[:])

                    # --- out-proj + normalize ---
                    for co in range(CO):
                        zps = ps_z.tile([128, CHUNK], F32, tag="zps")
                        for ci in range(CI):
                            nc.tensor.matmul(
                                zps[:], wo_sb[:, ci, co * 128:(co + 1) * 128],
                                y_sb[:, ci, :], start=(ci == 0), stop=(ci == CI - 1))
                        zst = zstp.tile([128, CHUNK], F32, tag="zst")
                        nc.vector.tensor_tensor(zst[:], zps[:], rb_sb[:],
                                                mybir.AluOpType.mult)
                        nc.sync.dma_start(zt[co * 128:(co + 1) * 128, rsl], zst[:])
    nc.compile()
    _NC_CACHE["nc"] = nc
    return nc


def make_in_maps(inputs):
    x = np.asarray(inputs["x"], dtype=np.float32)
    for bname in ("bq", "bk", "bv", "bo"):
        bval = np.asarray(inputs[bname])
        assert np.all(bval == 0.0), f"{bname} nonzero: unsupported fast path"
    wqt = np.ascontiguousarray(np.asarray(inputs["Wq"], np.float32).T)
    wkt = np.ascontiguousarray(np.asarray(inputs["Wk"], np.float32).T)
    wvt = np.ascontiguousarray(np.asarray(inputs["Wv"], np.float32).T)
    wot = np.ascontiguousarray(np.asarray(inputs["Wo"], np.float32).T)
    keyidx16 = (np.arange(T, dtype=np.float32).reshape(KB, 128).T + WINDOW
                ).copy()  # [128, KB]
    chunk_map = {0: (0, 3), 1: (1, 2)}  # slot 0 = denser chunk
    in_maps = []
    for core in range(N_CORES):
        b, h = divmod(core, 2)
        xt_b = np.ascontiguousarray(x[b].T)  # [C, T]
        ch0, ch1 = chunk_map[h]
        xtq = np.concatenate(
            [xt_b[:, ch0 * CHUNK:(ch0 + 1) * CHUNK],
             xt_b[:, ch1 * CHUNK:(ch1 + 1) * CHUNK]], axis=1)
        rowidx = np.concatenate(
            [np.arange(ch0 * CHUNK, (ch0 + 1) * CHUNK, dtype=np.float32),
             np.arange(ch1 * CHUNK, (ch1 + 1) * CHUNK, dtype=np.float32)])
        rowidxb = np.ascontiguousarray(
            np.broadcast_to(rowidx[None, :], (128, TOWN)))
        in_maps.append({
            "xt": xt_b, "xtq": np.ascontiguousarray(xtq),
            "wqt": wqt, "wkt": wkt, "wvt": wvt, "wot": wot,
            "keyidx16": keyidx16, "rowidxb": rowidxb,
        })
    return in_maps


def gather_output(results, dtype):
    out = np.empty((B, T, C), dtype=dtype)
    chunk_map = {0: (0, 3), 1: (1, 2)}
    for core in range(N_CORES):
        b, h = divmod(core, 2)
        y = results[core]["zt"].T  # [TOWN rows, C]
        ch0, ch1 = chunk_map[h]
        out[b, ch0 * CHUNK:(ch0 + 1) * CHUNK] = y[:CHUNK]
        out[b, ch1 * CHUNK:(ch1 + 1) * CHUNK] = y[CHUNK:]
    return out


def kernel(**inputs):
    nc = build()
    in_maps = make_in_maps(inputs)
    res = bass_utils.run_bass_kernel_spmd(nc, in_maps,
                                          core_ids=list(range(N_CORES)))
    return gather_output(res.results, np.asarray(inputs["x"]).dtype)
